# revision 1
# baseline (speedup 1.0000x reference)
"""Single-head causal attention (B=4, T=2048, C=2048, H=128) on 8 TRN2 cores.

Sharding: 2 cores per batch. T is split into 16 query tiles of 128 rows.
Core (2b + par) handles batch b and query tiles t in {par, par+2, ..., par+14}.
The query tile t = 2i-2+par (i = 1..8) is processed with a padded causal
key window of 2i key-tiles, so every core executes an identical program
(sum of windows = 72 [128x128] tile pairs); per-core inputs (gathered query
rows, two mask tiles) carry the asymmetry.

Per core on-device pipeline (all matmuls bf16 with fp32 PSUM accumulation):
  K^T = Wk.T @ x.T    [h=128, T]     (x.T supplied pre-transposed by host)
  Q^T = Wq.T @ xq.T   [h=128, 1024]  (xq = the core's 8 query tiles)
  V   = x @ Wv        [T, h] stored as 16 tiles [128, 129] with a ones column
  per q-tile: S^T(kt) = K^T(kt).T @ Q^T  -> exp(scale*S) on ACT -> bf16 A^T
              mask last two kt tiles; O/den = A^T.T @ [V|1] accumulated in PSUM
              out = O * reciprocal(den)
Softmax max-subtraction is skipped: scores are ~N(0,1)-scaled (|s| < ~6), so
exp is safely in fp32 range (matches the reference softmax exactly up to
rounding, softmax being shift-invariant).
"""

import numpy as np
import ml_dtypes

B, T, C, H = 4, 2048, 2048, 128
P = 128                 # tile edge
NCT = C // P            # 16 contraction chunks
NKT = T // P            # 16 key tiles
NQT = 8                 # query tiles per core
NQ = NQT * P            # 1024 query rows per core
N_CORES = 8
SCALE = float(H) ** -0.5
BF16 = ml_dtypes.bfloat16

_cache = {}


def _build():
    import concourse.bass as bass
    import concourse.mybir as mybir
    import concourse.tile as tile
    from concourse import bacc

    dt = mybir.dt
    nc = bacc.Bacc(
        "TRN2",
        target_bir_lowering=False,
        debug=False,
        enable_asserts=False,
        num_devices=N_CORES,
    )

    xkvT = nc.dram_tensor("xkvT", [C, T], dt.bfloat16, kind="ExternalInput").ap()
    xqT = nc.dram_tensor("xqT", [C, NQ], dt.bfloat16, kind="ExternalInput").ap()
    wq_d = nc.dram_tensor("wq", [C, H], dt.bfloat16, kind="ExternalInput").ap()
    wk_d = nc.dram_tensor("wk", [C, H], dt.bfloat16, kind="ExternalInput").ap()
    wv_d = nc.dram_tensor("wv", [C, H], dt.bfloat16, kind="ExternalInput").ap()
    # masks[:, 0:128] applies to the 2nd-to-last key tile of each window,
    # masks[:, 128:256] to the last. Layout [k_local, q_local].
    masks_d = nc.dram_tensor("masks", [P, 2 * P], dt.bfloat16, kind="ExternalInput").ap()
    out_d = nc.dram_tensor("o", [NQ, H], dt.float32, kind="ExternalOutput").ap()

    with tile.TileContext(nc) as tc:
        with (
            tc.tile_pool(name="persist", bufs=1) as persist,
            tc.tile_pool(name="ephem", bufs=3) as ephem,
            tc.tile_pool(name="outp", bufs=2) as outp,
        ):
            # ---- load everything into SBUF ----
            xkv_sb = persist.tile([P, NCT, T], dt.bfloat16)
            xq_sb = persist.tile([P, NCT, NQ], dt.bfloat16)
            wq_sb = persist.tile([P, NCT, H], dt.bfloat16)
            wk_sb = persist.tile([P, NCT, H], dt.bfloat16)
            wv_sb = persist.tile([P, NCT, H], dt.bfloat16)
            masks_sb = persist.tile([P, 2 * P], dt.bfloat16)
            k_sb = persist.tile([P, T], dt.bfloat16)          # K^T [h, T]
            q_sb = persist.tile([P, NQ], dt.bfloat16)         # Q^T [h, NQ]
            v_sb = persist.tile([P, NKT, H + 4], dt.bfloat16)  # V tiles + ones col

            nc.sync.dma_start(out=masks_sb[:], in_=masks_d[:])
            for w_sb, w_d in ((wq_sb, wq_d), (wk_sb, wk_d), (wv_sb, wv_d)):
                for j in range(NCT):
                    nc.sync.dma_start(
                        out=w_sb[:, j, :], in_=w_d[j * P:(j + 1) * P, :]
                    )
            for j in range(NCT):
                nc.sync.dma_start(out=xq_sb[:, j, :], in_=xqT[j * P:(j + 1) * P, :])
            for j in range(NCT):
                nc.sync.dma_start(out=xkv_sb[:, j, :], in_=xkvT[j * P:(j + 1) * P, :])

            nc.vector.memset(v_sb[:, :, H:H + 1], 1.0)

            # ---- projections: K^T and Q^T (j-outer, PSUM accumulate) ----
            with tc.tile_pool(name="pskq", bufs=1, space="PSUM") as pskq:
                ps_k = [pskq.tile([P, 512], dt.float32, name=f"psk{n}") for n in range(4)]
                ps_q = [pskq.tile([P, 512], dt.float32, name=f"psq{n}") for n in range(2)]
                for j in range(NCT):
                    st, sp = j == 0, j == NCT - 1
                    for n in range(4):
                        nc.tensor.matmul(
                            ps_k[n][:],
                            lhsT=wk_sb[:, j, :],
                            rhs=xkv_sb[:, j, 512 * n:512 * (n + 1)],
                            start=st, stop=sp,
                        )
                    for n in range(2):
                        nc.tensor.matmul(
                            ps_q[n][:],
                            lhsT=wq_sb[:, j, :],
                            rhs=xq_sb[:, j, 512 * n:512 * (n + 1)],
                            start=st, stop=sp,
                        )
                for n in range(4):
                    nc.vector.tensor_copy(k_sb[:, 512 * n:512 * (n + 1)], ps_k[n][:])
                for n in range(2):
                    nc.vector.tensor_copy(q_sb[:, 512 * n:512 * (n + 1)], ps_q[n][:])

            # ---- projection: V natural [k, h] (8 PSUM banks per half) ----
            with tc.tile_pool(name="psv", bufs=1, space="PSUM") as psv:
                ps_v = [psv.tile([P, H], dt.float32, name=f"psv{n}") for n in range(8)]
                for half in range(2):
                    for j in range(NCT):
                        st, sp = j == 0, j == NCT - 1
                        for n in range(8):
                            kt = 8 * half + n
                            nc.tensor.matmul(
                                ps_v[n][:],
                                lhsT=xkv_sb[:, j, kt * P:(kt + 1) * P],
                                rhs=wv_sb[:, j, :],
                                start=st, stop=sp,
                            )
                    for n in range(8):
                        kt = 8 * half + n
                        nc.vector.tensor_copy(v_sb[:, kt, 0:H], ps_v[n][:])

            # ---- attention per query tile ----
            with (
                tc.tile_pool(name="pss", bufs=4, space="PSUM") as pss,
                tc.tile_pool(name="pso", bufs=2, space="PSUM") as pso,
            ):
                for i in range(1, NQT + 1):
                    w = 2 * i  # key-tile window
                    qs = slice(P * (i - 1), P * i)
                    groups = [list(range(g, min(g + 4, w))) for g in range(0, w, 4)]

                    ps_o = pso.tile([P, H + 1], dt.float32, name="ps_o")
                    a_tiles = []
                    for kts in groups:
                        ps_s = pss.tile([P, 512], dt.float32, name="ps_s")
                        for u, kt in enumerate(kts):
                            nc.tensor.matmul(
                                ps_s[:, P * u:P * (u + 1)],
                                lhsT=k_sb[:, kt * P:(kt + 1) * P],
                                rhs=q_sb[:, qs],
                                start=True, stop=True,
                            )
                        a_sb = ephem.tile([P, 512], dt.bfloat16, name="a_sb")
                        n_el = P * len(kts)
                        nc.scalar.activation(
                            a_sb[:, :n_el], ps_s[:, :n_el],
                            mybir.ActivationFunctionType.Exp,
                            scale=SCALE,
                        )
                        a_tiles.append((a_sb, kts))

                    # mask the last two key tiles of the window
                    a_last, kts_last = a_tiles[-1]
                    r = len(kts_last)
                    nc.vector.tensor_mul(
                        a_last[:, P * (r - 2):P * r],
                        a_last[:, P * (r - 2):P * r],
                        masks_sb[:],
                    )

                    for a_sb, kts in a_tiles:
                        for u, kt in enumerate(kts):
                            nc.tensor.matmul(
                                ps_o[:],
                                lhsT=a_sb[:, P * u:P * (u + 1)],
                                rhs=v_sb[:, kt, 0:H + 1],
                                start=(kt == 0), stop=(kt == w - 1),
                            )

                    den = outp.tile([P, 1], dt.float32, name="den")
                    nc.vector.reciprocal(den[:], ps_o[:, H:H + 1])
                    o_sb = outp.tile([P, H], dt.float32, name="o_sb")
                    nc.vector.tensor_scalar_mul(o_sb[:], ps_o[:, 0:H], den[:])
                    nc.sync.dma_start(out=out_d[qs, :], in_=o_sb[:])

    nc.compile()
    return nc


def _core_tiles(core):
    par = core % 2
    return [2 * (i - 1) + par for i in range(1, NQT + 1)]


def _prep_inputs(x, Wq, Wk, Wv):
    """Build the 8 per-core input maps."""
    tri = (np.arange(P)[:, None] <= np.arange(P)[None, :])  # allowed: k_local <= q_local
    mask_even = np.concatenate([tri, np.zeros((P, P), bool)], axis=1)
    mask_odd = np.concatenate([np.ones((P, P), bool), tri], axis=1)

    wq_b = np.ascontiguousarray(Wq.astype(BF16))
    wk_b = np.ascontiguousarray(Wk.astype(BF16))
    wv_b = np.ascontiguousarray(Wv.astype(BF16))

    in_maps = []
    for core in range(N_CORES):
        b, par = core // 2, core % 2
        xT = np.ascontiguousarray(x[b].astype(BF16).T)  # [C, T]
        rows = np.concatenate([np.arange(P * t, P * t + P) for t in _core_tiles(core)])
        xqT = np.ascontiguousarray(xT[:, rows])
        mask = (mask_even if par == 0 else mask_odd).astype(BF16)
        in_maps.append({
            "xkvT": xT, "xqT": xqT,
            "wq": wq_b, "wk": wk_b, "wv": wv_b,
            "masks": np.ascontiguousarray(mask),
        })
    return in_maps


def _assemble(results):
    out = np.empty((B, T, H), np.float32)
    for core in range(N_CORES):
        o = results[core]["o"]
        for idx, t in enumerate(_core_tiles(core)):
            out[core // 2, P * t:P * (t + 1), :] = o[P * idx:P * (idx + 1), :]
    return out


def _run(inputs, trace=False, **spmd_kwargs):
    from concourse.bass_utils import run_bass_kernel_spmd

    if "nc" not in _cache:
        _cache["nc"] = _build()
    nc = _cache["nc"]
    in_maps = _prep_inputs(
        np.asarray(inputs["x"], np.float32),
        np.asarray(inputs["Wq"], np.float32),
        np.asarray(inputs["Wk"], np.float32),
        np.asarray(inputs["Wv"], np.float32),
    )
    res = run_bass_kernel_spmd(
        nc, in_maps, list(range(N_CORES)), trace=trace, **spmd_kwargs
    )
    return _assemble(res.results), res


def kernel(x, Wq, Wk, Wv):
    out, _ = _run({"x": x, "Wq": Wq, "Wk": Wk, "Wv": Wv})
    return out


# revision 3
# speedup vs baseline: 1.1878x; 1.1878x over previous
"""Single-head causal attention (B=4, T=2048, C=2048, H=128) on 8 TRN2 cores.

Sharding: 2 cores per batch. T is split into 16 query tiles of 128 rows.
Core (2b + par) handles batch b and query tiles t in {par, par+2, ..., par+14}.
Query tile class i (i = 1..8) is processed with a padded causal key window of
2i key tiles, so every core executes an identical program (72 [128x128]
attention tile pairs); per-core inputs carry the asymmetry.

Key-order permutation trick: the host reorders the T dimension of the per-core
x.T buffer as [own_1, sib_1, own_2, sib_2, ...] (own_i = the core's class-i
query tile, sib_i = the sibling core's). Attention sums are order-invariant
over keys, and the class-i key window is exactly the first 2i positions of
this order, so the program is position-based and identical across cores:
  - Q columns are the even positions (fixed offsets for every core),
  - the window's second-to-last position (even) is always the diagonal tile
    (constant triangular mask, built on device),
  - the last position (odd, the sibling tile) is all-allowed or all-masked
    depending only on core parity (a per-core [128,1] scalar input).

Per core on-device pipeline (all matmuls bf16, fp32 PSUM accumulation):
  K^T, V^T = W.T @ x.T   [128h, T]   (x.T chunks DMA'd and consumed in a
                                      pipelined j-loop; N=512 matmuls)
  Q^T = Wq.T @ x.T[:, even positions] (strided rhs AP)
  V tiles [k,h] via 16 PE transposes of V^T
  attention, kt-outer over two class halves (cols [0,512) and [512,1024)):
    S^T(kt) = K_kt.T @ Q^T[:, c0:]     (one N<=512 matmul)
    A = exp(scale * S^T) on ACT (softmax max-shift skipped: |s| < ~6)
    mask first 128-col block (tri if kt even, parity scalar if odd, only when
    the diagonal class is in this half)
    O^T[half] += V_kt.T(as lhsT) @ A   den[half] += ones.T @ A
  outputs: ot [128, 1024] f32, den [1, 1024] f32; host computes (ot/den).T
  and scatters rows back.
"""

import numpy as np
import ml_dtypes

B, T, C, H = 4, 2048, 2048, 128
P = 128                 # tile edge
NCT = C // P            # 16 contraction chunks
NKT = T // P            # 16 key tiles / positions
NQT = 8                 # query tile classes per core
NQ = NQT * P            # 1024 query rows per core
N_CORES = 8
SCALE = float(H) ** -0.5
BF16 = ml_dtypes.bfloat16

_cache = {}


def _build():
    import concourse.bass as bass
    import concourse.mybir as mybir
    import concourse.tile as tile
    from concourse import bacc
    from concourse.masks import make_identity, make_upper_triangular

    dt = mybir.dt
    nc = bacc.Bacc(
        "TRN2",
        target_bir_lowering=False,
        debug=False,
        enable_asserts=False,
        num_devices=N_CORES,
    )

    xkvT = nc.dram_tensor("xkvT", [C, T], dt.bfloat16, kind="ExternalInput").ap()
    wq_d = nc.dram_tensor("wq", [P, NCT, H], dt.bfloat16, kind="ExternalInput").ap()
    wk_d = nc.dram_tensor("wk", [P, NCT, H], dt.bfloat16, kind="ExternalInput").ap()
    wv_d = nc.dram_tensor("wv", [P, NCT, H], dt.bfloat16, kind="ExternalInput").ap()
    # parity scalar: 1.0 if the sibling (odd-position) key tile is allowed
    # (par=1 cores), 0.0 if masked (par=0 cores)
    odd_d = nc.dram_tensor("odd", [P, 1], dt.float32, kind="ExternalInput").ap()
    ot_d = nc.dram_tensor("ot", [H, NQ], dt.float32, kind="ExternalOutput").ap()
    den_d = nc.dram_tensor("den", [1, NQ], dt.float32, kind="ExternalOutput").ap()

    XJ = 2          # x chunks of XJ c-tiles each
    NG = NCT // XJ  # 8 pipelined load/compute chunks

    with tile.TileContext(nc) as tc:
        with (
            tc.tile_pool(name="persist", bufs=1) as persist,
            tc.tile_pool(name="ephem", bufs=3) as ephem,
            tc.tile_pool(name="outp", bufs=2) as outp,
        ):
            wq_sb = persist.tile([P, NCT, H], dt.bfloat16)
            wk_sb = persist.tile([P, NCT, H], dt.bfloat16)
            wv_sb = persist.tile([P, NCT, H], dt.bfloat16)
            odd_sb = persist.tile([P, 1], dt.float32)
            xkv_sb = persist.tile([P, NCT, T], dt.bfloat16)
            k_sb = persist.tile([P, T], dt.bfloat16)       # K^T [h, T]
            vt_sb = persist.tile([P, T], dt.bfloat16)      # V^T [h, T]
            v_sb = persist.tile([P, NKT, H], dt.bfloat16)  # V tiles [k, h]
            q_sb = persist.tile([P, NQ], dt.bfloat16)      # Q^T [h, NQ]
            ident = persist.tile([P, P], dt.bfloat16)
            tri = persist.tile([P, P], dt.bfloat16)        # 1 where k <= q
            ones_sb = persist.tile([P, 1], dt.bfloat16)

            nc.sync.dma_start(out=odd_sb[:], in_=odd_d[:])
            nc.sync.dma_start(out=wq_sb[:], in_=wq_d[:])
            nc.sync.dma_start(out=wk_sb[:], in_=wk_d[:])
            nc.sync.dma_start(out=wv_sb[:], in_=wv_d[:])
            make_identity(nc, ident[:])
            make_upper_triangular(nc, tri[:], val=1.0, diag=True)
            nc.vector.memset(ones_sb[:], 1.0)

            # ---- pipelined x load + K^T / V^T accumulation ----
            with tc.tile_pool(name="pskv", bufs=1, space="PSUM") as pskv:
                ps_k = [pskv.tile([P, 512], dt.float32, name=f"psk{n}") for n in range(4)]
                ps_vt = [pskv.tile([P, 512], dt.float32, name=f"psvt{n}") for n in range(4)]
                for g in range(NG):
                    eng = nc.sync if g % 2 == 0 else nc.scalar
                    eng.dma_start(
                        out=xkv_sb[:, XJ * g:XJ * (g + 1), :],
                        in_=xkvT[XJ * P * g:XJ * P * (g + 1), :].rearrange(
                            "(j p) t -> p j t", p=P
                        ),
                    )
                    for j in range(XJ * g, XJ * (g + 1)):
                        st, sp = j == 0, j == NCT - 1
                        for n in range(4):
                            nc.tensor.matmul(
                                ps_k[n][:],
                                lhsT=wk_sb[:, j, :],
                                rhs=xkv_sb[:, j, 512 * n:512 * (n + 1)],
                                start=st, stop=sp,
                            )
                        for n in range(4):
                            nc.tensor.matmul(
                                ps_vt[n][:],
                                lhsT=wv_sb[:, j, :],
                                rhs=xkv_sb[:, j, 512 * n:512 * (n + 1)],
                                start=st, stop=sp,
                            )
                for n in range(4):
                    nc.vector.tensor_copy(k_sb[:, 512 * n:512 * (n + 1)], ps_k[n][:])
                for n in range(4):
                    nc.vector.tensor_copy(vt_sb[:, 512 * n:512 * (n + 1)], ps_vt[n][:])

            # ---- Q^T (even positions of xkv) + V tiles (PE transpose) ----
            with (
                tc.tile_pool(name="psq", bufs=1, space="PSUM") as psq,
                tc.tile_pool(name="psvp", bufs=2, space="PSUM") as psvp,
            ):
                ps_q = [psq.tile([P, 512], dt.float32, name=f"psq{n}") for n in range(2)]
                for j in range(NCT):
                    st, sp = j == 0, j == NCT - 1
                    # even 128-col blocks of the pair-interleaved layout
                    xq_j = xkv_sb[:, j, :].rearrange("p (m two) -> p m two", two=2 * P)
                    for n in range(2):
                        nc.tensor.matmul(
                            ps_q[n][:],
                            lhsT=wq_sb[:, j, :],
                            rhs=xq_j[:, 4 * n:4 * (n + 1), 0:P],
                            start=st, stop=sp,
                        )
                for n in range(2):
                    nc.vector.tensor_copy(q_sb[:, 512 * n:512 * (n + 1)], ps_q[n][:])

                for kt in range(NKT):
                    ps_v = psvp.tile([P, P], dt.bfloat16, name="ps_v")
                    nc.tensor.transpose(
                        ps_v[:], vt_sb[:, kt * P:(kt + 1) * P], ident[:]
                    )
                    nc.vector.tensor_copy(v_sb[:, kt, :], ps_v[:])

            # ---- attention: kt-outer over two class halves ----
            with (
                tc.tile_pool(name="pss", bufs=3, space="PSUM") as pss,
                tc.tile_pool(name="psacc", bufs=1, space="PSUM") as psacc,
            ):
                ps_ot = [psacc.tile([P, 512], dt.float32, name=f"psot{h}") for h in range(2)]
                ps_den = [psacc.tile([1, 512], dt.float32, name=f"psden{h}") for h in range(2)]

                for half in range(2):
                    lo, hi = 512 * half, 512 * (half + 1)
                    nkt = 8 * (half + 1)
                    for kt in range(nkt):
                        c0 = max(P * (kt // 2), lo)
                        n = hi - c0
                        st, sp = kt == 0, kt == nkt - 1

                        ps_s = pss.tile([P, 512], dt.float32, name="ps_s")
                        nc.tensor.matmul(
                            ps_s[:, 0:n],
                            lhsT=k_sb[:, kt * P:(kt + 1) * P],
                            rhs=q_sb[:, c0:hi],
                            start=True, stop=True,
                        )
                        a_sb = ephem.tile([P, 512], dt.bfloat16, name="a_sb")
                        nc.scalar.activation(
                            a_sb[:, 0:n], ps_s[:, 0:n],
                            mybir.ActivationFunctionType.Exp,
                            scale=SCALE,
                        )
                        if c0 == P * (kt // 2):  # diagonal class is in this half
                            if kt % 2 == 0:
                                nc.vector.tensor_mul(
                                    a_sb[:, 0:P], a_sb[:, 0:P], tri[:]
                                )
                            else:
                                nc.vector.tensor_scalar_mul(
                                    a_sb[:, 0:P], a_sb[:, 0:P], odd_sb[:]
                                )
                        nc.tensor.matmul(
                            ps_ot[half][:, c0 - lo:512],
                            lhsT=v_sb[:, kt, :],
                            rhs=a_sb[:, 0:n],
                            start=st, stop=sp,
                        )
                        nc.tensor.matmul(
                            ps_den[half][:, c0 - lo:512],
                            lhsT=ones_sb[:],
                            rhs=a_sb[:, 0:n],
                            start=st, stop=sp,
                        )

                    ot_sb = outp.tile([P, 512], dt.float32, name="ot_sb")
                    nc.vector.tensor_copy(ot_sb[:], ps_ot[half][:])
                    nc.sync.dma_start(out=ot_d[:, lo:hi], in_=ot_sb[:])
                    den_sb = outp.tile([1, 512], dt.float32, name="den_sb")
                    nc.vector.tensor_copy(den_sb[:], ps_den[half][:])
                    nc.sync.dma_start(out=den_d[:, lo:hi], in_=den_sb[:])

    nc.compile()
    return nc


def _core_tiles(core):
    par = core % 2
    return [2 * (i - 1) + par for i in range(1, NQT + 1)]


def _prep_inputs(x, Wq, Wk, Wv):
    """Build the 8 per-core input maps."""
    def wshape(w):
        # [C, H] -> [128, NCT, H]: w_r[p, j, h] = w[j*128 + p, h]
        return np.ascontiguousarray(
            w.astype(BF16).reshape(NCT, P, H).transpose(1, 0, 2)
        )

    wq_b, wk_b, wv_b = wshape(Wq), wshape(Wk), wshape(Wv)
    x_bf = x.astype(BF16)

    in_maps = []
    for core in range(N_CORES):
        b, par = core // 2, core % 2
        # position -> global key tile: [own_1, sib_1, own_2, sib_2, ...]
        perm = []
        for m in range(NQT):
            perm += [2 * m + par, 2 * m + 1 - par]
        cols = np.concatenate([np.arange(P * t, P * t + P) for t in perm])
        xT = np.ascontiguousarray(x_bf[b].T[:, cols])
        odd = np.full((P, 1), float(par), np.float32)
        in_maps.append({
            "xkvT": xT,
            "wq": wq_b, "wk": wk_b, "wv": wv_b,
            "odd": np.ascontiguousarray(odd),
        })
    return in_maps


def _assemble(results):
    out = np.empty((B, T, H), np.float32)
    for core in range(N_CORES):
        r = results[core]
        o = (r["ot"] / r["den"]).T  # [NQ, H]
        for idx, t in enumerate(_core_tiles(core)):
            out[core // 2, P * t:P * (t + 1), :] = o[P * idx:P * (idx + 1), :]
    return out


def _run(inputs, trace=False, **spmd_kwargs):
    from concourse.bass_utils import run_bass_kernel_spmd

    if "nc" not in _cache:
        _cache["nc"] = _build()
    nc = _cache["nc"]
    in_maps = _prep_inputs(
        np.asarray(inputs["x"], np.float32),
        np.asarray(inputs["Wq"], np.float32),
        np.asarray(inputs["Wk"], np.float32),
        np.asarray(inputs["Wv"], np.float32),
    )
    res = run_bass_kernel_spmd(
        nc, in_maps, list(range(N_CORES)), trace=trace, **spmd_kwargs
    )
    return _assemble(res.results), res


def kernel(x, Wq, Wk, Wv):
    out, _ = _run({"x": x, "Wq": Wq, "Wk": Wk, "Wv": Wv})
    return out


# revision 4
# speedup vs baseline: 1.4177x; 1.1935x over previous
"""Single-head causal attention (B=4, T=2048, C=2048, H=128) on 8 TRN2 cores.

Sharding: 2 cores per batch. T is split into 16 query tiles of 128 rows.
Core (2b + par) handles batch b and query tiles t in {par, par+2, ..., par+14}.
Query tile class i (i = 1..8) is processed with a padded causal key window of
2i key tiles, so every core executes an identical program (72 [128x128]
attention tile pairs); per-core inputs carry the asymmetry.

Key-order permutation trick: the host reorders the T dimension of the per-core
x.T buffer as [own_1, sib_1, own_2, sib_2, ...] (own_i = the core's class-i
query tile, sib_i = the sibling core's). Attention sums are order-invariant
over keys, and the class-i key window is exactly the first 2i positions of
this order, so the program is position-based and identical across cores:
  - Q columns are the even positions (fixed offsets for every core),
  - the window's second-to-last position (even) is always the diagonal tile
    (constant triangular mask, built on device),
  - the last position (odd, the sibling tile) is all-allowed or all-masked
    depending only on core parity (a per-core [128,1] scalar input).

Per core on-device pipeline (all matmuls bf16, fp32 PSUM accumulation):
  phase 1 (DMA-paced): x.T arrives in 8 chunks; K^T, V^T accumulate per chunk
  phase 2: Q^T (even positions, strided rhs AP) + V tiles via PE transpose
  phase 3: attention, kt-outer over two class halves (cols [0,512), [512,1024)):
    S^T(kt) = K_kt.T @ Q^T[:, c0:]     (one N<=512 matmul)
    A = exp(scale * S^T) on ACT (softmax max-shift skipped: |s| < ~6)
    mask first 128-col block (tri if kt even, parity scalar if odd, only when
    the diagonal class is in this half)
    O^T[half] += V_kt(as lhsT) @ A ; den[half] += ones.T @ A
  outputs: ot [128, 1024] f32, den [1, 1024] f32; host computes (ot/den).T
  and scatters rows back.

PSUM is managed as one pool with 8 explicitly reused bank tags to avoid
pool-boundary serialization between phases.
"""

import numpy as np
import ml_dtypes

B, T, C, H = 4, 2048, 2048, 128
P = 128                 # tile edge
NCT = C // P            # 16 contraction chunks
NKT = T // P            # 16 key tiles / positions
NQT = 8                 # query tile classes per core
NQ = NQT * P            # 1024 query rows per core
N_CORES = 8
SCALE = float(H) ** -0.5
BF16 = ml_dtypes.bfloat16

_cache = {}


def _build():
    import concourse.bass as bass
    import concourse.mybir as mybir
    import concourse.tile as tile
    from concourse import bacc
    from concourse.masks import make_identity, make_upper_triangular

    dt = mybir.dt
    nc = bacc.Bacc(
        "TRN2",
        target_bir_lowering=False,
        debug=False,
        enable_asserts=False,
        num_devices=N_CORES,
    )

    xkvT = nc.dram_tensor("xkvT", [C, T], dt.bfloat16, kind="ExternalInput").ap()
    wq_d = nc.dram_tensor("wq", [P, NCT, H], dt.bfloat16, kind="ExternalInput").ap()
    wk_d = nc.dram_tensor("wk", [P, NCT, H], dt.bfloat16, kind="ExternalInput").ap()
    wv_d = nc.dram_tensor("wv", [P, NCT, H], dt.bfloat16, kind="ExternalInput").ap()
    # parity scalar: 1.0 if the sibling (odd-position) key tile is allowed
    # (par=1 cores), 0.0 if masked (par=0 cores)
    odd_d = nc.dram_tensor("odd", [P, 1], dt.float32, kind="ExternalInput").ap()
    ot_d = nc.dram_tensor("ot", [H, NQ], dt.float32, kind="ExternalOutput").ap()
    den_d = nc.dram_tensor("den", [1, NQ], dt.float32, kind="ExternalOutput").ap()

    XJ = 2          # c-tiles per x chunk
    NG = NCT // XJ  # 8 pipelined load/compute chunks

    with tile.TileContext(nc) as tc:
        with (
            tc.tile_pool(name="persist", bufs=1) as persist,
            tc.tile_pool(name="ephem", bufs=4) as ephem,
            tc.tile_pool(name="outp", bufs=2) as outp,
            tc.tile_pool(name="psum", bufs=1, space="PSUM") as psum,
        ):
            def bank(b, shape=(P, 512), dtype=dt.float32, name="pb"):
                return psum.tile(list(shape), dtype, tag=f"bank{b}", name=f"{name}{b}")

            wq_sb = persist.tile([P, NCT, H], dt.bfloat16)
            wk_sb = persist.tile([P, NCT, H], dt.bfloat16)
            wv_sb = persist.tile([P, NCT, H], dt.bfloat16)
            odd_sb = persist.tile([P, 1], dt.float32)
            xg_sb = [
                persist.tile([P, XJ, T], dt.bfloat16, name=f"xg{g}")
                for g in range(NG)
            ]
            k_sb = persist.tile([P, T], dt.bfloat16)       # K^T [h, T]
            vt_sb = persist.tile([P, T], dt.bfloat16)      # V^T [h, T]
            v_sb = persist.tile([P, NKT, H], dt.bfloat16)  # V tiles [k, h]
            q_sb = persist.tile([P, NQ], dt.bfloat16)      # Q^T [h, NQ]
            ident = persist.tile([P, P], dt.bfloat16)
            tri = persist.tile([P, P], dt.bfloat16)        # 1 where k <= q
            ones_sb = persist.tile([P, 1], dt.bfloat16)

            nc.sync.dma_start(out=odd_sb[:], in_=odd_d[:])
            nc.sync.dma_start(out=wq_sb[:], in_=wq_d[:])
            nc.sync.dma_start(out=wk_sb[:], in_=wk_d[:])
            nc.sync.dma_start(out=wv_sb[:], in_=wv_d[:])
            make_identity(nc, ident[:])
            make_upper_triangular(nc, tri[:], val=1.0, diag=True)
            nc.vector.memset(ones_sb[:], 1.0)

            # ---- phase 1: pipelined x load + K^T / V^T accumulation ----
            # banks 0-3: K accum; banks 4-7: V^T accum
            ps_k = [bank(n, name="psk") for n in range(4)]
            ps_vt = [bank(4 + n, name="psvt") for n in range(4)]
            for g in range(NG):
                nc.sync.dma_start(
                    out=xg_sb[g][:],
                    in_=xkvT[XJ * P * g:XJ * P * (g + 1), :].rearrange(
                        "(j p) t -> p j t", p=P
                    ),
                )
                for jj in range(XJ):
                    j = XJ * g + jj
                    st, sp = j == 0, j == NCT - 1
                    for n in range(4):
                        nc.tensor.matmul(
                            ps_k[n][:],
                            lhsT=wk_sb[:, j, :],
                            rhs=xg_sb[g][:, jj, 512 * n:512 * (n + 1)],
                            start=st, stop=sp,
                        )
                    for n in range(4):
                        nc.tensor.matmul(
                            ps_vt[n][:],
                            lhsT=wv_sb[:, j, :],
                            rhs=xg_sb[g][:, jj, 512 * n:512 * (n + 1)],
                            start=st, stop=sp,
                        )
            for n in range(4):
                nc.vector.tensor_copy(k_sb[:, 512 * n:512 * (n + 1)], ps_k[n][:])
            for n in range(4):
                nc.vector.tensor_copy(vt_sb[:, 512 * n:512 * (n + 1)], ps_vt[n][:])

            # ---- phase 2: Q^T (even positions) + V tiles (PE transpose) ----
            # banks 0-1: Q accum; banks 2-3: transpose ping-pong
            ps_q = [bank(n, name="psq") for n in range(2)]
            for j in range(NCT):
                g, jj = j // XJ, j % XJ
                st, sp = j == 0, j == NCT - 1
                xq_j = xg_sb[g][:, jj, :].rearrange("p (m two) -> p m two", two=2 * P)
                for n in range(2):
                    nc.tensor.matmul(
                        ps_q[n][:],
                        lhsT=wq_sb[:, j, :],
                        rhs=xq_j[:, 4 * n:4 * (n + 1), 0:P],
                        start=st, stop=sp,
                    )
            for n in range(2):
                nc.vector.tensor_copy(q_sb[:, 512 * n:512 * (n + 1)], ps_q[n][:])

            def v_transpose(kt):
                ps_v = bank(2 + kt % 2, shape=(P, P), dtype=dt.bfloat16, name="psv")
                nc.tensor.transpose(ps_v[:], vt_sb[:, kt * P:(kt + 1) * P], ident[:])
                nc.vector.tensor_copy(v_sb[:, kt, :], ps_v[:])

            for kt in range(8):
                v_transpose(kt)

            # ---- phase 3: attention, kt-outer over two class halves ----
            # banks 4-6: S tiles rotate; bank 7: OT half A; bank 2: den half A
            # bank 0: OT half B; bank 1: den half B (efter Q frees them)
            ps_ot = [bank(7, name="psotA"), bank(0, name="psotB")]
            ps_den = [
                bank(2, shape=(1, 512), name="psdenA"),
                bank(1, shape=(1, 512), name="psdenB"),
            ]

            def attention_half(half):
                lo, hi = 512 * half, 512 * (half + 1)
                nkt = 8 * (half + 1)
                for kt in range(nkt):
                    c0 = max(P * (kt // 2), lo)
                    n = hi - c0
                    st, sp = kt == 0, kt == nkt - 1

                    ps_s = bank(4 + kt % 3, name="pss")
                    nc.tensor.matmul(
                        ps_s[:, 0:n],
                        lhsT=k_sb[:, kt * P:(kt + 1) * P],
                        rhs=q_sb[:, c0:hi],
                        start=True, stop=True,
                    )
                    a_sb = ephem.tile([P, 512], dt.bfloat16, name="a_sb")
                    nc.scalar.activation(
                        a_sb[:, 0:n], ps_s[:, 0:n],
                        mybir.ActivationFunctionType.Exp,
                        scale=SCALE,
                    )
                    if c0 == P * (kt // 2):  # diagonal class is in this half
                        if kt % 2 == 0:
                            nc.vector.tensor_mul(a_sb[:, 0:P], a_sb[:, 0:P], tri[:])
                        else:
                            nc.vector.tensor_scalar_mul(
                                a_sb[:, 0:P], a_sb[:, 0:P], odd_sb[:]
                            )
                    nc.tensor.matmul(
                        ps_ot[half][:, c0 - lo:512],
                        lhsT=v_sb[:, kt, :],
                        rhs=a_sb[:, 0:n],
                        start=st, stop=sp,
                    )
                    nc.tensor.matmul(
                        ps_den[half][:, c0 - lo:512],
                        lhsT=ones_sb[:],
                        rhs=a_sb[:, 0:n],
                        start=st, stop=sp,
                    )

                ot_sb = outp.tile([P, 512], dt.float32, name="ot_sb")
                nc.vector.tensor_copy(ot_sb[:], ps_ot[half][:])
                nc.sync.dma_start(out=ot_d[:, lo:hi], in_=ot_sb[:])
                den_sb = outp.tile([1, 512], dt.float32, name="den_sb")
                nc.vector.tensor_copy(den_sb[:], ps_den[half][:])
                nc.sync.dma_start(out=den_d[:, lo:hi], in_=den_sb[:])

            attention_half(0)
            for kt in range(8, 16):
                v_transpose(kt)
            attention_half(1)

    nc.compile()
    return nc


def _core_tiles(core):
    par = core % 2
    return [2 * (i - 1) + par for i in range(1, NQT + 1)]


def _prep_inputs(x, Wq, Wk, Wv):
    """Build the 8 per-core input maps."""
    def wshape(w):
        # [C, H] -> [128, NCT, H]: w_r[p, j, h] = w[j*128 + p, h]
        return np.ascontiguousarray(
            w.astype(BF16).reshape(NCT, P, H).transpose(1, 0, 2)
        )

    wq_b, wk_b, wv_b = wshape(Wq), wshape(Wk), wshape(Wv)
    x_bf = x.astype(BF16)

    in_maps = []
    for core in range(N_CORES):
        b, par = core // 2, core % 2
        # position -> global key tile: [own_1, sib_1, own_2, sib_2, ...]
        perm = []
        for m in range(NQT):
            perm += [2 * m + par, 2 * m + 1 - par]
        cols = np.concatenate([np.arange(P * t, P * t + P) for t in perm])
        xT = np.ascontiguousarray(x_bf[b].T[:, cols])
        odd = np.full((P, 1), float(par), np.float32)
        in_maps.append({
            "xkvT": xT,
            "wq": wq_b, "wk": wk_b, "wv": wv_b,
            "odd": np.ascontiguousarray(odd),
        })
    return in_maps


def _assemble(results):
    out = np.empty((B, T, H), np.float32)
    for core in range(N_CORES):
        r = results[core]
        o = (r["ot"] / r["den"]).T  # [NQ, H]
        for idx, t in enumerate(_core_tiles(core)):
            out[core // 2, P * t:P * (t + 1), :] = o[P * idx:P * (idx + 1), :]
    return out


def _run(inputs, trace=False, **spmd_kwargs):
    from concourse.bass_utils import run_bass_kernel_spmd

    if "nc" not in _cache:
        _cache["nc"] = _build()
    nc = _cache["nc"]
    in_maps = _prep_inputs(
        np.asarray(inputs["x"], np.float32),
        np.asarray(inputs["Wq"], np.float32),
        np.asarray(inputs["Wk"], np.float32),
        np.asarray(inputs["Wv"], np.float32),
    )
    res = run_bass_kernel_spmd(
        nc, in_maps, list(range(N_CORES)), trace=trace, **spmd_kwargs
    )
    return _assemble(res.results), res


def kernel(x, Wq, Wk, Wv):
    out, _ = _run({"x": x, "Wq": Wq, "Wk": Wk, "Wv": Wv})
    return out


# revision 6
# speedup vs baseline: 1.5372x; 1.0843x over previous
"""Single-head causal attention (B=4, T=2048, C=2048, H=128) on 8 TRN2 cores.

Sharding: 2 cores per batch. T is split into 16 query tiles of 128 rows.
Core (2b + par) handles batch b and query tiles t in {par, par+2, ..., par+14}.
Query tile class i (i = 1..8) is processed with a padded causal key window of
2i key tiles, so every core executes an identical program (72 [128x128]
attention tile pairs); per-core inputs carry the asymmetry.

Key-order permutation trick: the host reorders the T dimension of the per-core
x.T buffer as [own_1, sib_1, own_2, sib_2, ...] (own_i = the core's class-i
query tile, sib_i = the sibling core's). Attention sums are order-invariant
over keys, and the class-i key window is exactly the first 2i positions of
this order, so the program is position-based and identical across cores:
  - Q columns are the even positions (fixed offsets for every core),
  - the window's second-to-last position (even) is always the diagonal tile
    (constant triangular mask, built on device),
  - the last position (odd, the sibling tile) is all-allowed or all-masked
    depending only on core parity (a per-core [128,1] scalar input).

Per core on-device pipeline (all matmuls bf16, fp32 PSUM accumulation):
  phase 1 (DMA-paced): x.T arrives in 8 chunks; K^T, V^T accumulate per chunk
  phase 2: Q^T (even positions, strided rhs AP) + V tiles via PE transpose
  phase 3: attention, kt-outer over two class halves (cols [0,512), [512,1024)):
    S^T(kt) = K_kt.T @ Q^T[:, c0:]     (one N<=512 matmul)
    A = exp(scale * S^T) on ACT (softmax max-shift skipped: |s| < ~6)
    mask first 128-col block (tri if kt even, parity scalar if odd, only when
    the diagonal class is in this half)
    O^T[half] += V_kt(as lhsT) @ A ; den[half] += ones.T @ A
  outputs: ot [128, 1024] f32, den [1, 1024] f32; host computes (ot/den).T
  and scatters rows back.

PSUM is managed as one pool with 8 explicitly reused bank tags to avoid
pool-boundary serialization between phases.
"""

import numpy as np
import ml_dtypes

B, T, C, H = 4, 2048, 2048, 128
P = 128                 # tile edge
NCT = C // P            # 16 contraction chunks
NKT = T // P            # 16 key tiles / positions
NQT = 8                 # query tile classes per core
NQ = NQT * P            # 1024 query rows per core
N_CORES = 8
SCALE = float(H) ** -0.5
BF16 = ml_dtypes.bfloat16

_cache = {}


def _build():
    import concourse.bass as bass
    import concourse.mybir as mybir
    import concourse.tile as tile
    from concourse import bacc
    from concourse.masks import make_identity, make_upper_triangular

    dt = mybir.dt
    nc = bacc.Bacc(
        "TRN2",
        target_bir_lowering=False,
        debug=False,
        enable_asserts=False,
        num_devices=N_CORES,
    )

    xkvT = nc.dram_tensor("xkvT", [C, T], dt.bfloat16, kind="ExternalInput").ap()
    wq_d = nc.dram_tensor("wq", [P, NCT, H], dt.bfloat16, kind="ExternalInput").ap()
    wk_d = nc.dram_tensor("wk", [P, NCT, H], dt.bfloat16, kind="ExternalInput").ap()
    wv_d = nc.dram_tensor("wv", [P, NCT, H], dt.bfloat16, kind="ExternalInput").ap()
    # parity scalar: 1.0 if the sibling (odd-position) key tile is allowed
    # (par=1 cores), 0.0 if masked (par=0 cores)
    odd_d = nc.dram_tensor("odd", [P, 1], dt.float32, kind="ExternalInput").ap()
    ot_d = nc.dram_tensor("ot", [H, NQ], dt.float32, kind="ExternalOutput").ap()
    den_d = nc.dram_tensor("den", [1, NQ], dt.float32, kind="ExternalOutput").ap()

    XJ = 2          # c-tiles per x chunk
    NG = NCT // XJ  # 8 pipelined load/compute chunks

    with tile.TileContext(nc) as tc:
        with (
            tc.tile_pool(name="persist", bufs=1) as persist,
            tc.tile_pool(name="ephem", bufs=4) as ephem,
            tc.tile_pool(name="outp", bufs=2) as outp,
            tc.tile_pool(name="psum", bufs=1, space="PSUM") as psum,
        ):
            def bank(b, shape=(P, 512), dtype=dt.float32, name="pb"):
                return psum.tile(list(shape), dtype, tag=f"bank{b}", name=f"{name}{b}")

            wq_sb = persist.tile([P, NCT, H], dt.bfloat16)
            wk_sb = persist.tile([P, NCT, H], dt.bfloat16)
            wv_sb = persist.tile([P, NCT, H], dt.bfloat16)
            odd_sb = persist.tile([P, 1], dt.float32)
            xg_sb = [
                persist.tile([P, XJ, T], dt.bfloat16, name=f"xg{g}")
                for g in range(NG)
            ]
            k_sb = persist.tile([P, T], dt.bfloat16)       # K^T [h, T]
            vt_sb = persist.tile([P, T], dt.bfloat16)      # V^T [h, T]
            v_sb = persist.tile([P, NKT, H], dt.bfloat16)  # V tiles [k, h]
            q_sb = persist.tile([P, NQ], dt.bfloat16)      # Q^T [h, NQ]
            ident = persist.tile([P, P], dt.bfloat16)
            tri = persist.tile([P, P], dt.bfloat16)        # 1 where k <= q
            ones_sb = persist.tile([P, 1], dt.bfloat16)

            nc.sync.dma_start(out=wk_sb[:], in_=wk_d[:])
            nc.sync.dma_start(out=wv_sb[:], in_=wv_d[:])
            nc.sync.dma_start(out=wq_sb[:], in_=wq_d[:])
            nc.sync.dma_start(out=odd_sb[:], in_=odd_d[:])
            make_identity(nc, ident[:])
            make_upper_triangular(nc, tri[:], val=1.0, diag=True)
            nc.vector.memset(ones_sb[:], 1.0)
            # preload the ACT exp table off the attention critical path
            warm_sb = persist.tile([P, 1], dt.float32)
            nc.scalar.activation(
                warm_sb[:], ones_sb[:], mybir.ActivationFunctionType.Exp
            )

            # ---- phase 1: pipelined x load + K^T / V^T accumulation ----
            # banks 0-3: K accum; banks 4-7: V^T accum
            ps_k = [bank(n, name="psk") for n in range(4)]
            ps_vt = [bank(4 + n, name="psvt") for n in range(4)]
            for g in range(NG):
                # scalar (ACT) HWDGE ring: runs in parallel with the weight
                # loads queued on the sync ring
                nc.scalar.dma_start(
                    out=xg_sb[g][:],
                    in_=xkvT[XJ * P * g:XJ * P * (g + 1), :].rearrange(
                        "(j p) t -> p j t", p=P
                    ),
                )
                for jj in range(XJ):
                    j = XJ * g + jj
                    st, sp = j == 0, j == NCT - 1
                    for n in range(4):
                        nc.tensor.matmul(
                            ps_k[n][:],
                            lhsT=wk_sb[:, j, :],
                            rhs=xg_sb[g][:, jj, 512 * n:512 * (n + 1)],
                            start=st, stop=sp,
                        )
                    for n in range(4):
                        nc.tensor.matmul(
                            ps_vt[n][:],
                            lhsT=wv_sb[:, j, :],
                            rhs=xg_sb[g][:, jj, 512 * n:512 * (n + 1)],
                            start=st, stop=sp,
                        )
            for n in range(4):
                nc.vector.tensor_copy(k_sb[:, 512 * n:512 * (n + 1)], ps_k[n][:])
            for n in range(4):
                nc.vector.tensor_copy(vt_sb[:, 512 * n:512 * (n + 1)], ps_vt[n][:])

            # ---- phase 2: Q^T (even positions) + V tiles (PE transpose) ----
            # banks 0-1: Q accum; banks 2-3: transpose ping-pong
            ps_q = [bank(n, name="psq") for n in range(2)]
            for j in range(NCT):
                g, jj = j // XJ, j % XJ
                st, sp = j == 0, j == NCT - 1
                xq_j = xg_sb[g][:, jj, :].rearrange("p (m two) -> p m two", two=2 * P)
                for n in range(2):
                    nc.tensor.matmul(
                        ps_q[n][:],
                        lhsT=wq_sb[:, j, :],
                        rhs=xq_j[:, 4 * n:4 * (n + 1), 0:P],
                        start=st, stop=sp,
                    )
            for n in range(2):
                nc.vector.tensor_copy(q_sb[:, 512 * n:512 * (n + 1)], ps_q[n][:])

            def v_transpose(kt):
                ps_v = bank(2 + kt % 2, shape=(P, P), dtype=dt.bfloat16, name="psv")
                nc.tensor.transpose(ps_v[:], vt_sb[:, kt * P:(kt + 1) * P], ident[:])
                nc.vector.tensor_copy(v_sb[:, kt, :], ps_v[:])

            for kt in range(8):
                v_transpose(kt)

            # ---- phase 3: attention, kt-outer over two class halves ----
            # banks 4-6: S tiles rotate; bank 7: OT half A; bank 2: den half A
            # bank 0: OT half B; bank 1: den half B (efter Q frees them)
            ps_ot = [bank(7, name="psotA"), bank(0, name="psotB")]
            ps_den = [
                bank(2, shape=(1, 512), name="psdenA"),
                bank(1, shape=(1, 512), name="psdenB"),
            ]

            def attention_half(half):
                lo, hi = 512 * half, 512 * (half + 1)
                nkt = 8 * (half + 1)
                for kt in range(nkt):
                    c0 = max(P * (kt // 2), lo)
                    n = hi - c0
                    st, sp = kt == 0, kt == nkt - 1

                    ps_s = bank(4 + kt % 3, name="pss")
                    nc.tensor.matmul(
                        ps_s[:, 0:n],
                        lhsT=k_sb[:, kt * P:(kt + 1) * P],
                        rhs=q_sb[:, c0:hi],
                        start=True, stop=True,
                    )
                    a_sb = ephem.tile([P, 512], dt.bfloat16, name="a_sb")
                    nc.scalar.activation(
                        a_sb[:, 0:n], ps_s[:, 0:n],
                        mybir.ActivationFunctionType.Exp,
                        scale=SCALE,
                    )
                    if c0 == P * (kt // 2):  # diagonal class is in this half
                        if kt % 2 == 0:
                            nc.vector.tensor_mul(a_sb[:, 0:P], a_sb[:, 0:P], tri[:])
                        else:
                            nc.vector.tensor_scalar_mul(
                                a_sb[:, 0:P], a_sb[:, 0:P], odd_sb[:]
                            )
                    nc.tensor.matmul(
                        ps_ot[half][:, c0 - lo:512],
                        lhsT=v_sb[:, kt, :],
                        rhs=a_sb[:, 0:n],
                        start=st, stop=sp,
                    )
                    nc.tensor.matmul(
                        ps_den[half][:, c0 - lo:512],
                        lhsT=ones_sb[:],
                        rhs=a_sb[:, 0:n],
                        start=st, stop=sp,
                    )

                ot_sb = outp.tile([P, 512], dt.float32, name="ot_sb")
                nc.vector.tensor_copy(ot_sb[:], ps_ot[half][:])
                nc.sync.dma_start(out=ot_d[:, lo:hi], in_=ot_sb[:])
                den_sb = outp.tile([1, 512], dt.float32, name="den_sb")
                nc.vector.tensor_copy(den_sb[:], ps_den[half][:])
                nc.sync.dma_start(out=den_d[:, lo:hi], in_=den_sb[:])

            attention_half(0)
            for kt in range(8, 16):
                v_transpose(kt)
            attention_half(1)

    nc.compile()
    return nc


def _core_tiles(core):
    par = core % 2
    return [2 * (i - 1) + par for i in range(1, NQT + 1)]


def _prep_inputs(x, Wq, Wk, Wv):
    """Build the 8 per-core input maps."""
    def wshape(w):
        # [C, H] -> [128, NCT, H]: w_r[p, j, h] = w[j*128 + p, h]
        return np.ascontiguousarray(
            w.astype(BF16).reshape(NCT, P, H).transpose(1, 0, 2)
        )

    wq_b, wk_b, wv_b = wshape(Wq), wshape(Wk), wshape(Wv)
    x_bf = x.astype(BF16)

    in_maps = []
    for core in range(N_CORES):
        b, par = core // 2, core % 2
        # position -> global key tile: [own_1, sib_1, own_2, sib_2, ...]
        perm = []
        for m in range(NQT):
            perm += [2 * m + par, 2 * m + 1 - par]
        cols = np.concatenate([np.arange(P * t, P * t + P) for t in perm])
        xT = np.ascontiguousarray(x_bf[b].T[:, cols])
        odd = np.full((P, 1), float(par), np.float32)
        in_maps.append({
            "xkvT": xT,
            "wq": wq_b, "wk": wk_b, "wv": wv_b,
            "odd": np.ascontiguousarray(odd),
        })
    return in_maps


def _assemble(results):
    out = np.empty((B, T, H), np.float32)
    for core in range(N_CORES):
        r = results[core]
        o = (r["ot"] / r["den"]).T  # [NQ, H]
        for idx, t in enumerate(_core_tiles(core)):
            out[core // 2, P * t:P * (t + 1), :] = o[P * idx:P * (idx + 1), :]
    return out


def _run(inputs, trace=False, **spmd_kwargs):
    from concourse.bass_utils import run_bass_kernel_spmd

    if "nc" not in _cache:
        _cache["nc"] = _build()
    nc = _cache["nc"]
    in_maps = _prep_inputs(
        np.asarray(inputs["x"], np.float32),
        np.asarray(inputs["Wq"], np.float32),
        np.asarray(inputs["Wk"], np.float32),
        np.asarray(inputs["Wv"], np.float32),
    )
    res = run_bass_kernel_spmd(
        nc, in_maps, list(range(N_CORES)), trace=trace, **spmd_kwargs
    )
    return _assemble(res.results), res


def kernel(x, Wq, Wk, Wv):
    out, _ = _run({"x": x, "Wq": Wq, "Wk": Wk, "Wv": Wv})
    return out
